# revision 1
# baseline (speedup 1.0000x reference)
"""Trainium2 Bass kernel for LocalWindowAttention.

Reference computation (see problem):
  x [B=2, S=8192, D=1024] -> q,k,v = x @ {wq,wk,wv}.T ; RoPE(q,k) with
  positions arange(16) per 16-token window; non-causal softmax attention
  within each window; out = attn @ wo.T.

Strategy: data-parallel over tokens (8 cores x 2048 tokens; windows of 16
never cross shard boundaries).  Per core, one pass over 128-token tiles:

  xT tile -> Q,K,V projections (fp32r matmuls, N=512) -> RoPE on Q,K (DVE,
  reading PSUM; the rotate-half index swap is carried by the output AP and
  a pre-swapped sin table) -> PE-transpose Q,K to [d, tok] -> per head,
  scoresT = KT_h^T-contraction (a 128x128 block = 8 windows; off-window
  entries killed by a multiplicative block mask after exp) -> exp on ACT ->
  PV matmul with a ones column appended to V so the softmax denominator Z
  lands in the same PSUM tile -> normalize with a per-quad broadcast
  multiply -> PE-transpose attn to [d, tok] (fp32r) -> WO matmuls -> out.

PSUM rule learned the hard way: matmuls writing the same PSUM bank must
use one K-partition range (mixing offsets 0/64 within a bank crashes the
device), hence head quads group heads of equal parity.
"""

import os
import sys

import numpy as np

for _p in ("/root/.axon_site/_ro/trn_rl_repo", "/opt/trn_rl_repo"):
    if os.path.isdir(_p) and _p not in sys.path:
        sys.path.append(_p)

import concourse.bass as bass
import concourse.tile as tile
from concourse import bacc, mybir
from concourse.bass_utils import run_bass_kernel_spmd

F32 = mybir.dt.float32
F32R = mybir.dt.float32r
AF = mybir.ActivationFunctionType

DIM = 1024
NHEADS = 16
HDIM = 64
WIN = 16
B, S = 2, 8192
NCORES = 8
TOK_TOTAL = B * S
TOK_PER_CORE = TOK_TOTAL // NCORES

# heads grouped so each scores PSUM bank sees one K-partition offset
HEAD_QUADS = [[0, 2, 4, 6], [8, 10, 12, 14], [1, 3, 5, 7], [9, 11, 13, 15]]


def _emit(nc, tokens, repeat=1, loop_trips=0):
    ntiles = tokens // 128

    xt = nc.dram_tensor("xt", [DIM, tokens], F32R, kind="ExternalInput")
    wqt = nc.dram_tensor("wqt", [DIM, DIM], F32R, kind="ExternalInput")
    wkt = nc.dram_tensor("wkt", [DIM, DIM], F32R, kind="ExternalInput")
    wvt = nc.dram_tensor("wvt", [DIM, DIM], F32R, kind="ExternalInput")
    wot = nc.dram_tensor("wot", [DIM, DIM], F32R, kind="ExternalInput")
    cosd = nc.dram_tensor("cosd", [128, HDIM], F32, kind="ExternalInput")
    sinswd = nc.dram_tensor("sinswd", [128, HDIM], F32, kind="ExternalInput")
    mskd = nc.dram_tensor("mskd", [128, 128], F32, kind="ExternalInput")
    idnd = nc.dram_tensor("idnd", [128, 128], F32R, kind="ExternalInput")
    out = nc.dram_tensor("out", [tokens, DIM], F32, kind="ExternalOutput")

    xtr = xt.rearrange("(c p) t -> p c t", p=128)
    wrs = [
        w.rearrange("(c p) o -> p c o", p=128) for w in (wqt, wkt, wvt, wot)
    ]

    with tile.TileContext(nc) as tc:
        with (
            tc.tile_pool(name="wpool", bufs=1) as wpool,
            tc.tile_pool(name="cpool", bufs=1) as cpool,
            tc.tile_pool(name="xpool", bufs=2) as xpool,
            tc.tile_pool(name="rpool", bufs=2) as rpool,
            tc.tile_pool(name="qkpool", bufs=1) as qkpool,
            tc.tile_pool(name="qkr", bufs=1) as qkr,
            tc.tile_pool(name="vpool", bufs=2) as vpool,
            tc.tile_pool(name="tpool", bufs=2) as tpool,
            tc.tile_pool(name="epool", bufs=6) as epool,
            tc.tile_pool(name="zpool", bufs=4) as zpool,
            tc.tile_pool(name="apool", bufs=1) as apool,
            tc.tile_pool(name="opool", bufs=2) as opool,
            tc.tile_pool(name="pp", bufs=3, space="PSUM") as pp,
            tc.tile_pool(name="ts", bufs=3, space="PSUM") as ts,
            tc.tile_pool(name="pa", bufs=2, space="PSUM") as pa,
        ):
            w_sbs = []
            for i, wr in enumerate(wrs):
                w_sb = wpool.tile([128, 8, DIM], F32R, tag=f"w{i}")
                nc.sync.dma_start(w_sb[:], wr[:])
                w_sbs.append(w_sb)
            wq_sb, wk_sb, wv_sb, wo_sb = w_sbs

            cos_sb = cpool.tile([128, HDIM], F32, tag="cos")
            nc.sync.dma_start(cos_sb[:], cosd[:])
            sinsw_sb = cpool.tile([128, HDIM], F32, tag="sinsw")
            nc.sync.dma_start(sinsw_sb[:], sinswd[:])
            msk_sb = cpool.tile([128, 128], F32, tag="msk")
            nc.sync.dma_start(msk_sb[:], mskd[:])
            idn_sb = cpool.tile([128, 128], F32R, tag="idn")
            nc.sync.dma_start(idn_sb[:], idnd[:])

            def body():
                for ip in range(ntiles // 2):
                    # pair-wide transposed Q/K: [d, 256 tokens]
                    qt2 = qkpool.tile([128, 8, 256], F32R, tag="qt")
                    kt2 = qkpool.tile([128, 8, 256], F32R, tag="kt")
                    vo_ts = []
                    at_ts = []
                    for sub in range(2):
                        it = 2 * ip + sub
                        t0 = it * 128
                        tsl = slice(t0, t0 + 128)
                        xt_t = xpool.tile([128, 8, 128], F32R, tag="xin")
                        nc.sync.dma_start(xt_t[:], xtr[:, :, tsl])

                        # --- Q, K projections + rope (natural layout) ---
                        qr_t = qkr.tile([128, DIM], F32R, tag="qr")
                        kr_t = qkr.tile([128, DIM], F32R, tag="kr")
                        for w_sb, dst in ((wq_sb, qr_t), (wk_sb, kr_t)):
                            for hf in range(2):
                                sl = slice(512 * hf, 512 * hf + 512)
                                ps = pp.tile([128, 512], F32, tag="pp")
                                for c in range(8):
                                    nc.tensor.matmul(
                                        ps[:],
                                        xt_t[:, c, :],
                                        w_sb[:, c, sl],
                                        start=(c == 0),
                                        stop=(c == 7),
                                    )
                                # t1[rh(d)] = ps[d]*sinsw[d] (swap via out AP)
                                t1 = rpool.tile([128, 512], F32, tag="rt1")
                                t1rh = bass.AP(
                                    tensor=t1.tensor,
                                    offset=t1.offset + 32,
                                    ap=[t1.ap[0], [64, 8], [-32, 2], [1, 32]],
                                )
                                sbc = bass.AP(
                                    tensor=sinsw_sb.tensor,
                                    offset=sinsw_sb.offset,
                                    ap=[sinsw_sb.ap[0], [0, 8], [32, 2], [1, 32]],
                                )
                                nc.vector.tensor_mul(
                                    t1rh,
                                    ps[:].rearrange(
                                        "p (h s j) -> p h s j", h=8, s=2
                                    ),
                                    sbc,
                                )
                                cbc = bass.AP(
                                    tensor=cos_sb.tensor,
                                    offset=cos_sb.offset,
                                    ap=[cos_sb.ap[0], [0, 8], [1, HDIM]],
                                )
                                t2 = rpool.tile([128, 512], F32, tag="rt2")
                                nc.vector.tensor_mul(
                                    t2[:].rearrange("p (g d) -> p g d", g=8),
                                    ps[:].rearrange("p (g d) -> p g d", g=8),
                                    cbc,
                                )
                                nc.vector.tensor_add(dst[:, sl], t1[:], t2[:])

                        # --- V projection -> vo [128, 16, 65] + ones col ---
                        vo_t = vpool.tile([128, NHEADS, HDIM + 1], F32, tag="vo")
                        for hf in range(2):
                            ps = pp.tile([128, 512], F32, tag="pp")
                            for c in range(8):
                                nc.tensor.matmul(
                                    ps[:],
                                    xt_t[:, c, :],
                                    wv_sb[:, c, 512 * hf : 512 * hf + 512],
                                    start=(c == 0),
                                    stop=(c == 7),
                                )
                            nc.scalar.copy(
                                vo_t[:, 8 * hf : 8 * hf + 8, 0:HDIM],
                                ps[:].rearrange("p (h d) -> p h d", h=8),
                            )
                        nc.gpsimd.memset(vo_t[:, :, HDIM : HDIM + 1], 1.0)
                        vo_ts.append(vo_t)

                        # --- PE transposes: Qr,Kr -> halves of qt2/kt2 ---
                        for src_t, dstt in ((qr_t, qt2), (kr_t, kt2)):
                            for qd in range(2):
                                pt_ = ts.tile([128, 512], F32R, tag="ts")
                                for c4 in range(4):
                                    c = 4 * qd + c4
                                    nc.tensor.transpose(
                                        pt_[:, 128 * c4 : 128 * c4 + 128],
                                        src_t[:, 128 * c : 128 * c + 128],
                                        idn_sb[:],
                                    )
                                nc.scalar.copy(
                                    dstt[
                                        :,
                                        4 * qd : 4 * qd + 4,
                                        128 * sub : 128 * sub + 128,
                                    ],
                                    pt_[:].rearrange("p (c t) -> p c t", c=4),
                                )

                    # --- scores (f32r, N=256 over both sub-tiles) + softmax
                    #     numerator; PV + normalize per sub-tile ---
                    attn_ts = [
                        apool.tile([128, DIM], F32R, tag="attn0", name="attn0"),
                        apool.tile([128, DIM], F32R, tag="attn1", name="attn1"),
                    ]
                    mbc = bass.AP(
                        tensor=msk_sb.tensor,
                        offset=msk_sb.offset,
                        ap=[msk_sb.ap[0], [0, 2], [1, 128]],
                    )
                    for qd in range(4):
                        et_q = []
                        for h4 in range(4):
                            h = 4 * qd + h4
                            po = (h % 2) * 64
                            ch = h // 2
                            ps_s = ts.tile([128, 512], F32, tag="ts")
                            for tau in range(2):
                                nc.tensor.matmul(
                                    ps_s[:, 256 * tau : 256 * tau + 256],
                                    kt2[
                                        po : po + 64,
                                        ch,
                                        128 * tau : 128 * tau + 128,
                                    ],
                                    qt2[po : po + 64, ch, :],
                                    start=True,
                                    stop=True,
                                )
                            # exp of the two useful 128-col blocks (tau==sub)
                            etc = epool.tile([128, 2, 128], F32, tag="et")
                            esrc = bass.AP(
                                tensor=ps_s.tensor,
                                offset=ps_s.offset,
                                ap=[ps_s.ap[0], [384, 2], [1, 128]],
                            )
                            nc.scalar.activation(
                                etc[:], esrc, AF.Exp, scale=0.125
                            )
                            nc.gpsimd.tensor_mul(etc[:], etc[:], mbc)
                            et_q.append(etc)
                        for sub in range(2):
                            pa_t = pa.tile([128, 4 * (HDIM + 1)], F32, tag="pa")
                            for h4 in range(4):
                                nc.tensor.matmul(
                                    pa_t[:, 65 * h4 : 65 * h4 + 65],
                                    et_q[h4][:, sub, :],
                                    vo_ts[sub][:, 4 * qd + h4, :],
                                    start=True,
                                    stop=True,
                                )
                            ziv = zpool.tile([128, 4], F32, tag="zi")
                            zsrc = bass.AP(
                                tensor=pa_t.tensor,
                                offset=pa_t.offset + HDIM,
                                ap=[pa_t.ap[0], [HDIM + 1, 4]],
                            )
                            nc.vector.reciprocal(ziv[:], zsrc)
                            srca = bass.AP(
                                tensor=pa_t.tensor,
                                offset=pa_t.offset,
                                ap=[pa_t.ap[0], [HDIM + 1, 4], [1, HDIM]],
                            )
                            zbc = bass.AP(
                                tensor=ziv.tensor,
                                offset=ziv.offset,
                                ap=[ziv.ap[0], [1, 4], [0, HDIM]],
                            )
                            nc.vector.tensor_mul(
                                attn_ts[sub][
                                    :, 256 * qd : 256 * qd + 256
                                ].rearrange("p (h d) -> p h d", h=4),
                                srca,
                                zbc,
                            )

                    # --- transpose attn -> attnT; WO matmuls -> out ---
                    for sub in range(2):
                        it = 2 * ip + sub
                        t0 = it * 128
                        tsl = slice(t0, t0 + 128)
                        attn_t = attn_ts[sub]
                        at_t = tpool.tile([128, 8, 128], F32R, tag="at")
                        for qd in range(2):
                            pt_ = ts.tile([128, 512], F32R, tag="ts")
                            for c4 in range(4):
                                c = 4 * qd + c4
                                nc.tensor.transpose(
                                    pt_[:, 128 * c4 : 128 * c4 + 128],
                                    attn_t[:, 128 * c : 128 * c + 128],
                                    idn_sb[:],
                                )
                            nc.vector.tensor_copy(
                                at_t[:, 4 * qd : 4 * qd + 4, :],
                                pt_[:].rearrange("p (c t) -> p c t", c=4),
                            )
                        o_t = opool.tile([128, DIM], F32, tag="o")
                        for hf in range(2):
                            ps = pp.tile([128, 512], F32, tag="pp")
                            for c in range(8):
                                nc.tensor.matmul(
                                    ps[:],
                                    at_t[:, c, :],
                                    wo_sb[:, c, 512 * hf : 512 * hf + 512],
                                    start=(c == 0),
                                    stop=(c == 7),
                                )
                            nc.scalar.copy(
                                o_t[:, 512 * hf : 512 * hf + 512], ps[:]
                            )
                        nc.sync.dma_start(out[tsl, :], o_t[:])

            if loop_trips:
                with tc.For_i(0, loop_trips, 1):
                    body()
            else:
                for _rep in range(repeat):
                    body()
    return nc


_PROGRAMS = {}


def build_program(tokens=TOK_PER_CORE, repeat=1, loop_trips=0):
    key = (tokens, repeat, loop_trips)
    if key not in _PROGRAMS:
        nc = bacc.Bacc("TRN2")
        _emit(nc, tokens, repeat, loop_trips)
        nc.compile()
        _PROGRAMS[key] = nc
    return _PROGRAMS[key]


def host_tables(rope_freqs):
    freqs = np.asarray(rope_freqs, dtype=np.float32)[:WIN]  # [16, 32]
    cos = np.cos(freqs)
    sin = np.sin(freqs)
    pos = np.arange(128) % WIN
    cos_ext = np.zeros((128, HDIM), dtype=np.float32)
    sinsw = np.zeros((128, HDIM), dtype=np.float32)
    cos_ext[:, 0:32] = cos[pos]
    cos_ext[:, 32:64] = cos[pos]
    # pre-swapped sin table: t1[rh(d)] = ps[d]*sinsw[d] must equal
    # rh(ps)[d']*sin_ext[d'] with sin_ext = [-sin, +sin]; so
    # sinsw = rh(sin_ext) = [+sin, -sin]
    sinsw[:, 0:32] = sin[pos]
    sinsw[:, 32:64] = -sin[pos]
    p = np.arange(128)
    c = np.arange(128)
    msk = (p[:, None] // WIN == c[None, :] // WIN).astype(np.float32)
    idn = np.eye(128, dtype=np.float32)
    return cos_ext, sinsw, np.ascontiguousarray(msk), idn


def make_in_maps(x, rope_freqs, wq, wk, wv, wo, tokens=TOK_PER_CORE, ncores=NCORES):
    x = np.asarray(x, dtype=np.float32)
    xf = x.reshape(-1, DIM)
    xT = np.ascontiguousarray(xf.T)  # [DIM, TOK_TOTAL]
    wqt = np.ascontiguousarray(np.asarray(wq, dtype=np.float32).T)
    wkt = np.ascontiguousarray(np.asarray(wk, dtype=np.float32).T)
    wvt = np.ascontiguousarray(np.asarray(wv, dtype=np.float32).T)
    wot = np.ascontiguousarray(np.asarray(wo, dtype=np.float32).T)
    cos_ext, sinsw, msk, idn = host_tables(rope_freqs)
    maps = []
    for c in range(ncores):
        sl = slice(c * tokens, (c + 1) * tokens)
        maps.append(
            {
                "xt": np.ascontiguousarray(xT[:, sl]),
                "wqt": wqt,
                "wkt": wkt,
                "wvt": wvt,
                "wot": wot,
                "cosd": cos_ext,
                "sinswd": sinsw,
                "mskd": msk,
                "idnd": idn,
            }
        )
    return maps


def kernel(x, rope_freqs, wq, wk, wv, wo):
    nc = build_program(TOK_PER_CORE, 1)
    maps = make_in_maps(x, rope_freqs, wq, wk, wv, wo)
    res = run_bass_kernel_spmd(nc, maps, core_ids=list(range(NCORES)))
    outs = [res.results[c]["out"] for c in range(NCORES)]
    full = np.concatenate(outs, axis=0)  # [TOK_TOTAL, DIM]
    return full.reshape(B, S, DIM).astype(np.float32)



# revision 3
# speedup vs baseline: 383.1900x; 383.1900x over previous
"""Trainium2 Bass kernel for LocalWindowAttention (bf16 megatile version).

Reference computation:
  x [B=2, S=8192, D=1024] -> q,k,v = x @ {wq,wk,wv}.T ; RoPE(q,k) with
  positions arange(16) per 16-token window; non-causal softmax attention
  within each window; out = attn @ wo.T.

Strategy: data-parallel over tokens (8 cores x 2048 tokens; windows never
cross shard boundaries).  Per core, 4 megatiles of 512 tokens, everything
bf16 on the wires with f32 PSUM accumulation:

  Q,K are projected directly in TRANSPOSED layout [d, tok] (weight chunk
  stationary, xT moving, N=512) so no PE transpose is needed before the
  scores matmul.  RoPE in transposed layout: rotate-half is a signed
  128x128 block-permutation matmul (P @ QT), then two broadcast DVE
  multiplies (cosT/sinT tables [128, 16] broadcast along (dblock, window))
  and one add.  V is projected in natural layout [tok, d] with a ones
  column appended so the PV matmul also yields the softmax denominator Z.
  Scores per head: 4 diagonal [128,128] blocks packed into one PSUM bank
  -> one Exp (ACT, scale=1/8) -> one same-window mask multiply (Pool).
  PV per (qt-block, 4-head group) -> reciprocal + broadcast normalize ->
  attn natural bf16 -> PE transpose -> WO matmuls -> out (bf16, host
  upcasts).

  The whole body sits inside a hardware For_i loop; `repeat` changes only
  the trip-count immediate, keeping program size constant so the harness's
  wall(r=R)-wall(r=1) delta measures actual execution, not NEFF size.

PSUM bank budget (8 banks): pp(proj+WO) bufs=2, sp(scores) bufs=2,
mix(rh+attnT) bufs=2, pa(PV) bufs=2.  Scores banks alternate head parity
with bufs=2 so each bank only ever sees one K-partition range (mixing
0/64 offsets within a bank crashes the device).
"""

import os
import sys

import numpy as np
import ml_dtypes

for _p in ("/root/.axon_site/_ro/trn_rl_repo", "/opt/trn_rl_repo"):
    if os.path.isdir(_p) and _p not in sys.path:
        sys.path.append(_p)

import concourse.bass as bass
import concourse.tile as tile
from concourse import bacc, mybir
from concourse.bass_utils import run_bass_kernel_spmd

F32 = mybir.dt.float32
BF16 = mybir.dt.bfloat16
AF = mybir.ActivationFunctionType
BF = ml_dtypes.bfloat16

DIM = 1024
NHEADS = 16
HDIM = 64
WIN = 16
B, S = 2, 8192
NCORES = 8
TOK_TOTAL = B * S
TOK_PER_CORE = TOK_TOTAL // NCORES
MEGA = 512  # tokens per megatile


def _emit(nc, tokens, repeat=1, use_loop=True, phases=9, staggered=False):
    nmega = tokens // MEGA

    xt = nc.dram_tensor("xt", [DIM, tokens], BF16, kind="ExternalInput")
    wqt = nc.dram_tensor("wqt", [DIM, DIM], BF16, kind="ExternalInput")
    wkt = nc.dram_tensor("wkt", [DIM, DIM], BF16, kind="ExternalInput")
    wvt = nc.dram_tensor("wvt", [DIM, DIM], BF16, kind="ExternalInput")
    wot = nc.dram_tensor("wot", [DIM, DIM], BF16, kind="ExternalInput")
    cosd = nc.dram_tensor("cosd", [128, WIN], BF16, kind="ExternalInput")
    sind = nc.dram_tensor("sind", [128, WIN], BF16, kind="ExternalInput")
    psgd = nc.dram_tensor("psgd", [128, 128], BF16, kind="ExternalInput")
    mskd = nc.dram_tensor("mskd", [128, 128], BF16, kind="ExternalInput")
    idnd = nc.dram_tensor("idnd", [128, 128], BF16, kind="ExternalInput")
    out = nc.dram_tensor("out", [tokens, DIM], BF16, kind="ExternalOutput")

    xtr = xt.rearrange("(c p) t -> p c t", p=128)
    outr = out.rearrange("(n p) d -> p n d", p=128)
    wrs = [w.rearrange("(c p) o -> p c o", p=128) for w in (wqt, wkt, wvt, wot)]

    with tile.TileContext(nc) as tc:
        with (
            tc.tile_pool(name="wpool", bufs=1) as wpool,
            tc.tile_pool(name="cpool", bufs=1) as cpool,
            tc.tile_pool(name="xpool", bufs=2) as xpool,
            tc.tile_pool(name="qkc", bufs=1) as qkc,
            tc.tile_pool(name="qks", bufs=1) as qks,
            tc.tile_pool(name="qkr", bufs=1) as qkr,
            tc.tile_pool(name="vpool", bufs=4) as vpool,
            tc.tile_pool(name="epool", bufs=16) as epool,
            tc.tile_pool(name="zpool", bufs=4) as zpool,
            tc.tile_pool(name="apool", bufs=4) as apool,
            tc.tile_pool(name="tpool", bufs=2) as tpool,
            tc.tile_pool(name="opool", bufs=2) as opool,
            tc.tile_pool(name="pp", bufs=2, space="PSUM") as pp,
            tc.tile_pool(name="sp", bufs=2, space="PSUM") as sp,
            tc.tile_pool(name="mix", bufs=2, space="PSUM") as mix,
            tc.tile_pool(name="pa", bufs=2, space="PSUM") as pa,
        ):
            w_sbs = []
            for i, wr in enumerate(wrs):
                w_sb = wpool.tile([128, 8, DIM], BF16, tag=f"w{i}")
                nc.sync.dma_start(w_sb[:], wr[:])
                w_sbs.append(w_sb)
            wq_sb, wk_sb, wv_sb, wo_sb = w_sbs

            cos_sb = cpool.tile([128, WIN], BF16, tag="cos")
            nc.sync.dma_start(cos_sb[:], cosd[:])
            sin_sb = cpool.tile([128, WIN], BF16, tag="sin")
            nc.sync.dma_start(sin_sb[:], sind[:])
            psg_sb = cpool.tile([128, 128], BF16, tag="psg")
            nc.sync.dma_start(psg_sb[:], psgd[:])
            msk_sb = cpool.tile([128, 128], BF16, tag="msk")
            nc.sync.dma_start(msk_sb[:], mskd[:])
            idn_sb = cpool.tile([128, 128], BF16, tag="idn")
            nc.sync.dma_start(idn_sb[:], idnd[:])

            # broadcast APs for the rope tables: free dims (dblock, win, j)
            cos_bc8 = bass.AP(
                tensor=cos_sb.tensor,
                offset=cos_sb.offset,
                ap=[cos_sb.ap[0], [0, 8], [0, MEGA // WIN], [1, WIN]],
            )
            sin_bc1 = bass.AP(
                tensor=sin_sb.tensor,
                offset=sin_sb.offset,
                ap=[sin_sb.ap[0], [0, MEGA // WIN], [1, WIN]],
            )
            msk_bc = bass.AP(
                tensor=msk_sb.tensor,
                offset=msk_sb.offset,
                ap=[msk_sb.ap[0], [0, 4], [1, 128]],
            )

            def megatile(it):
                t0 = it * MEGA

                x_mt = xpool.tile([128, 8, MEGA], BF16, tag="x")
                nc.sync.dma_start(x_mt[:], xtr[:, :, t0 : t0 + MEGA])

                # --- Q,K transposed projection + rope ---
                qr_ts = []
                for w_sb, nm in ((wq_sb, "q"), (wk_sb, "k")):
                    qc_t = qkc.tile([128, 8, MEGA], BF16, tag=f"{nm}c")
                    for db in range(8):
                        ps = pp.tile([128, MEGA], F32, tag="pp")
                        for c in range(8):
                            nc.tensor.matmul(
                                ps[:],
                                w_sb[:, c, 128 * db : 128 * db + 128],
                                x_mt[:, c, :],
                                start=(c == 0),
                                stop=(c == 7),
                            )
                        nc.scalar.copy(qc_t[:, db, :], ps[:])
                    qs_t = qks.tile([128, 8, MEGA], BF16, tag=f"{nm}s")
                    for db in range(8):
                        rps = mix.tile([128, MEGA], F32, tag="mix")
                        nc.tensor.matmul(
                            rps[:], psg_sb[:], qc_t[:, db, :],
                            start=True, stop=True,
                        )
                        nc.vector.tensor_mul(qs_t[:, db, :], rps[:], sin_bc1)
                    qr_t = qkr.tile([128, 8, MEGA], BF16, tag=f"{nm}r")
                    nc.vector.tensor_mul(qr_t[:], qc_t[:], cos_bc8)
                    nc.vector.tensor_add(qr_t[:], qr_t[:], qs_t[:])
                    qr_ts.append(qr_t)
                qr_t, kr_t = qr_ts

                if phases < 2:
                    o_t = opool.tile([128, 4, DIM], BF16, tag="o")
                    nc.gpsimd.memset(o_t[:], 0.0)
                    nc.sync.dma_start(outr[:, 4 * it : 4 * it + 4, :], o_t[:])
                    return
                # --- V natural projection with ones column ---
                vo_ts = []
                for tb in range(4):
                    vo_t = vpool.tile([128, NHEADS, HDIM + 1], BF16, tag="vo")
                    for hf in range(2):
                        ps = pp.tile([128, MEGA], F32, tag="pp")
                        for c in range(8):
                            nc.tensor.matmul(
                                ps[:],
                                x_mt[:, c, 128 * tb : 128 * tb + 128],
                                wv_sb[:, c, 512 * hf : 512 * hf + 512],
                                start=(c == 0),
                                stop=(c == 7),
                            )
                        nc.scalar.copy(
                            vo_t[:, 8 * hf : 8 * hf + 8, 0:HDIM],
                            ps[:].rearrange("p (h d) -> p h d", h=8),
                        )
                    nc.gpsimd.memset(vo_t[:, :, HDIM : HDIM + 1], 1.0)
                    vo_ts.append(vo_t)

                if phases < 3:
                    o_t = opool.tile([128, 4, DIM], BF16, tag="o")
                    nc.gpsimd.memset(o_t[:], 0.0)
                    nc.sync.dma_start(outr[:, 4 * it : 4 * it + 4, :], o_t[:])
                    return
                # --- scores + softmax numerator per head ---
                et_ts = []
                for h in range(NHEADS):
                    po = 64 * (h % 2)
                    ch = h // 2
                    sps = sp.tile([128, 512], F32, tag="sp")
                    for b in range(4):
                        nc.tensor.matmul(
                            sps[:, 128 * b : 128 * b + 128],
                            kr_t[po : po + 64, ch, 128 * b : 128 * b + 128],
                            qr_t[po : po + 64, ch, 128 * b : 128 * b + 128],
                            start=True,
                            stop=True,
                        )
                    et_t = epool.tile([128, 512], BF16, tag="et")
                    nc.scalar.activation(et_t[:], sps[:], AF.Exp, scale=0.125)
                    nc.gpsimd.tensor_mul(et_t[:], et_t[:], msk_bc)
                    et_ts.append(et_t)

                if phases < 4:
                    o_t = opool.tile([128, 4, DIM], BF16, tag="o")
                    nc.gpsimd.memset(o_t[:], 0.0)
                    nc.sync.dma_start(outr[:, 4 * it : 4 * it + 4, :], o_t[:])
                    return
                # --- PV + normalize -> attn natural bf16 ---
                attn_ts = []
                for b in range(4):
                    attn_t = apool.tile([128, DIM], BF16, tag="attn")
                    for g in range(4):
                        pa_t = pa.tile([128, 4 * (HDIM + 1)], F32, tag="pa")
                        for h4 in range(4):
                            h = 4 * g + h4
                            nc.tensor.matmul(
                                pa_t[:, 65 * h4 : 65 * h4 + 65],
                                et_ts[h][:, 128 * b : 128 * b + 128],
                                vo_ts[b][:, h, :],
                                start=True,
                                stop=True,
                            )
                        ziv = zpool.tile([128, 4], F32, tag="zi")
                        zsrc = bass.AP(
                            tensor=pa_t.tensor,
                            offset=pa_t.offset + HDIM,
                            ap=[pa_t.ap[0], [HDIM + 1, 4]],
                        )
                        nc.vector.reciprocal(ziv[:], zsrc)
                        srca = bass.AP(
                            tensor=pa_t.tensor,
                            offset=pa_t.offset,
                            ap=[pa_t.ap[0], [HDIM + 1, 4], [1, HDIM]],
                        )
                        zbc = bass.AP(
                            tensor=ziv.tensor,
                            offset=ziv.offset,
                            ap=[ziv.ap[0], [1, 4], [0, HDIM]],
                        )
                        nc.vector.tensor_mul(
                            attn_t[:, 256 * g : 256 * g + 256].rearrange(
                                "p (h d) -> p h d", h=4
                            ),
                            srca,
                            zbc,
                        )
                    attn_ts.append(attn_t)

                if phases < 5:
                    o_t = opool.tile([128, 4, DIM], BF16, tag="o")
                    nc.gpsimd.memset(o_t[:], 0.0)
                    nc.sync.dma_start(outr[:, 4 * it : 4 * it + 4, :], o_t[:])
                    return
                # --- attn transpose + WO -> out ---
                o_t = opool.tile([128, 4, DIM], BF16, tag="o")
                for b in range(4):
                    at_t = tpool.tile([128, 8, 128], BF16, tag="at")
                    for qd in range(2):
                        tp = mix.tile([128, 512], BF16, tag="mix")
                        for c4 in range(4):
                            c = 4 * qd + c4
                            nc.tensor.transpose(
                                tp[:, 128 * c4 : 128 * c4 + 128],
                                attn_ts[b][:, 128 * c : 128 * c + 128],
                                idn_sb[:],
                            )
                        nc.scalar.copy(
                            at_t[:, 4 * qd : 4 * qd + 4, :],
                            tp[:].rearrange("p (c t) -> p c t", c=4),
                        )
                    for hf in range(2):
                        ps = pp.tile([128, MEGA], F32, tag="pp")
                        for c in range(8):
                            nc.tensor.matmul(
                                ps[:],
                                at_t[:, c, :],
                                wo_sb[:, c, 512 * hf : 512 * hf + 512],
                                start=(c == 0),
                                stop=(c == 7),
                            )
                        nc.scalar.copy(o_t[:, b, 512 * hf : 512 * hf + 512], ps[:])
                nc.sync.dma_start(outr[:, 4 * it : 4 * it + 4, :], o_t[:])

            if use_loop:
                with tc.For_i(
                    0, repeat, 1,
                    hint_engines=(mybir.EngineType.PE,),
                    staggered_reset=staggered,
                ):
                    for it in range(nmega):
                        megatile(it)
            else:
                for _ in range(repeat):
                    for it in range(nmega):
                        megatile(it)
    return nc


_PROGRAMS = {}


def build_program(tokens=TOK_PER_CORE, repeat=1, loop_trips=0, use_loop=True,
                  phases=9, staggered=False):
    # loop_trips kept for interface compat; repeat IS the hardware trip count
    if loop_trips:
        repeat = loop_trips
    key = (tokens, repeat, use_loop, phases, staggered)
    if key not in _PROGRAMS:
        nc = bacc.Bacc("TRN2")
        _emit(nc, tokens, repeat, use_loop, phases, staggered)
        nc.compile()
        _PROGRAMS[key] = nc
    return _PROGRAMS[key]


def host_tables(rope_freqs):
    freqs = np.asarray(rope_freqs, dtype=np.float32)[:WIN]  # [16, 32]
    p = np.arange(128)
    # transposed-layout rope tables [128 (d%32 pattern), 16 (pos)]
    cosT = np.cos(freqs.T[p % 32])  # [128, 16]
    sinT = np.sin(freqs.T[p % 32])
    # signed rotate-half: rh(q)[d] = -q[d+32] (d%64<32), +q[d-32] (else)
    P = np.zeros((128, 128), dtype=np.float32)
    for blk in range(2):
        o = 64 * blk
        for d in range(32):
            P[o + d, o + d + 32] = -1.0
            P[o + d + 32, o + d] = 1.0
    psgT = np.ascontiguousarray(P.T)
    c = np.arange(128)
    msk = (p[:, None] // WIN == c[None, :] // WIN).astype(np.float32)
    idn = np.eye(128, dtype=np.float32)
    return (
        cosT.astype(BF), sinT.astype(BF), psgT.astype(BF),
        np.ascontiguousarray(msk).astype(BF), idn.astype(BF),
    )


def make_in_maps(x, rope_freqs, wq, wk, wv, wo, tokens=TOK_PER_CORE, ncores=NCORES):
    x = np.asarray(x, dtype=np.float32)
    xf = x.reshape(-1, DIM)
    xT = np.ascontiguousarray(xf.T).astype(BF)  # [DIM, TOK_TOTAL]
    wqt = np.ascontiguousarray(np.asarray(wq, dtype=np.float32).T).astype(BF)
    wkt = np.ascontiguousarray(np.asarray(wk, dtype=np.float32).T).astype(BF)
    wvt = np.ascontiguousarray(np.asarray(wv, dtype=np.float32).T).astype(BF)
    wot = np.ascontiguousarray(np.asarray(wo, dtype=np.float32).T).astype(BF)
    cosT, sinT, psgT, msk, idn = host_tables(rope_freqs)
    maps = []
    for c in range(ncores):
        sl = slice(c * tokens, (c + 1) * tokens)
        maps.append(
            {
                "xt": np.ascontiguousarray(xT[:, sl]),
                "wqt": wqt,
                "wkt": wkt,
                "wvt": wvt,
                "wot": wot,
                "cosd": cosT,
                "sind": sinT,
                "psgd": psgT,
                "mskd": msk,
                "idnd": idn,
            }
        )
    return maps


def kernel(x, rope_freqs, wq, wk, wv, wo):
    nc = build_program(TOK_PER_CORE, 1)
    maps = make_in_maps(x, rope_freqs, wq, wk, wv, wo)
    res = run_bass_kernel_spmd(nc, maps, core_ids=list(range(NCORES)))
    outs = [np.asarray(res.results[c]["out"]) for c in range(NCORES)]
    full = np.concatenate(outs, axis=0)  # [TOK_TOTAL, DIM] bf16
    return full.astype(np.float32).reshape(B, S, DIM)


# revision 4
# speedup vs baseline: 386.7970x; 1.0094x over previous
"""Trainium2 Bass kernel for LocalWindowAttention (bf16 megatile version).

Reference computation:
  x [B=2, S=8192, D=1024] -> q,k,v = x @ {wq,wk,wv}.T ; RoPE(q,k) with
  positions arange(16) per 16-token window; non-causal softmax attention
  within each window; out = attn @ wo.T.

Strategy: data-parallel over tokens (8 cores x 2048 tokens; windows never
cross shard boundaries).  Per core, 4 megatiles of 512 tokens, everything
bf16 on the wires with f32 PSUM accumulation:

  Q,K are projected directly in TRANSPOSED layout [d, tok] (weight chunk
  stationary, xT moving, N=512) so no PE transpose is needed before the
  scores matmul.  RoPE in transposed layout: rotate-half is a signed
  128x128 block-permutation matmul (P @ QT), then two broadcast DVE
  multiplies (cosT/sinT tables [128, 16] broadcast along (dblock, window))
  and one add.  V is projected in natural layout [tok, d] with a ones
  column appended so the PV matmul also yields the softmax denominator Z.
  Scores per head: 4 diagonal [128,128] blocks packed into one PSUM bank
  -> one Exp (ACT, scale=1/8) -> one same-window mask multiply (Pool).
  PV per (qt-block, 4-head group) -> reciprocal + broadcast normalize ->
  attn natural bf16 -> PE transpose -> WO matmuls -> out (bf16, host
  upcasts).

  The whole body sits inside a hardware For_i loop; `repeat` changes only
  the trip-count immediate, keeping program size constant so the harness's
  wall(r=R)-wall(r=1) delta measures actual execution, not NEFF size.

PSUM bank budget (8 banks): pp(proj+WO) bufs=2, sp(scores) bufs=2,
mix(rh+attnT) bufs=2, pa(PV) bufs=2.  Scores banks alternate head parity
with bufs=2 so each bank only ever sees one K-partition range (mixing
0/64 offsets within a bank crashes the device).
"""

import os
import sys

import numpy as np
import ml_dtypes

for _p in ("/root/.axon_site/_ro/trn_rl_repo", "/opt/trn_rl_repo"):
    if os.path.isdir(_p) and _p not in sys.path:
        sys.path.append(_p)

import concourse.bass as bass
import concourse.tile as tile
from concourse import bacc, mybir
from concourse.bass_utils import run_bass_kernel_spmd

F32 = mybir.dt.float32
BF16 = mybir.dt.bfloat16
AF = mybir.ActivationFunctionType
BF = ml_dtypes.bfloat16

DIM = 1024
NHEADS = 16
HDIM = 64
WIN = 16
B, S = 2, 8192
NCORES = 8
TOK_TOTAL = B * S
TOK_PER_CORE = TOK_TOTAL // NCORES
MEGA = 512  # tokens per megatile


def _emit(nc, tokens, repeat=1, use_loop=True, phases=9, staggered=False):
    nmega = tokens // MEGA

    # x pre-tiled on host: [mega, partition, kchunk, tok] so each partition's
    # per-megatile slice is one contiguous 8KB DMA descriptor
    xt = nc.dram_tensor(
        "xt", [tokens // MEGA, 128, 8, MEGA], BF16, kind="ExternalInput"
    )
    wqt = nc.dram_tensor("wqt", [DIM, DIM], BF16, kind="ExternalInput")
    wkt = nc.dram_tensor("wkt", [DIM, DIM], BF16, kind="ExternalInput")
    wvt = nc.dram_tensor("wvt", [DIM, DIM], BF16, kind="ExternalInput")
    wot = nc.dram_tensor("wot", [DIM, DIM], BF16, kind="ExternalInput")
    cosd = nc.dram_tensor("cosd", [128, WIN], BF16, kind="ExternalInput")
    sind = nc.dram_tensor("sind", [128, WIN], BF16, kind="ExternalInput")
    psgd = nc.dram_tensor("psgd", [128, 128], BF16, kind="ExternalInput")
    mskd = nc.dram_tensor("mskd", [128, 128], BF16, kind="ExternalInput")
    idnd = nc.dram_tensor("idnd", [128, 128], BF16, kind="ExternalInput")
    # out partition-major: [partition, tokblock, d]; host reassembles token
    # order (token = tokblock*128 + partition)
    out = nc.dram_tensor(
        "out", [128, tokens // 128, DIM], BF16, kind="ExternalOutput"
    )
    wrs = [w.rearrange("(c p) o -> p c o", p=128) for w in (wqt, wkt, wvt, wot)]

    with tile.TileContext(nc) as tc:
        with (
            tc.tile_pool(name="wpool", bufs=1) as wpool,
            tc.tile_pool(name="cpool", bufs=1) as cpool,
            tc.tile_pool(name="xpool", bufs=2) as xpool,
            tc.tile_pool(name="qkc", bufs=1) as qkc,
            tc.tile_pool(name="qks", bufs=1) as qks,
            tc.tile_pool(name="qkr", bufs=1) as qkr,
            tc.tile_pool(name="vpool", bufs=4) as vpool,
            tc.tile_pool(name="epool", bufs=16) as epool,
            tc.tile_pool(name="zpool", bufs=4) as zpool,
            tc.tile_pool(name="apool", bufs=4) as apool,
            tc.tile_pool(name="tpool", bufs=2) as tpool,
            tc.tile_pool(name="opool", bufs=2) as opool,
            tc.tile_pool(name="pp", bufs=2, space="PSUM") as pp,
            tc.tile_pool(name="sp", bufs=2, space="PSUM") as sp,
            tc.tile_pool(name="mix", bufs=2, space="PSUM") as mix,
            tc.tile_pool(name="pa", bufs=2, space="PSUM") as pa,
        ):
            w_sbs = []
            for i, wr in enumerate(wrs):
                w_sb = wpool.tile([128, 8, DIM], BF16, tag=f"w{i}")
                nc.sync.dma_start(w_sb[:], wr[:])
                w_sbs.append(w_sb)
            wq_sb, wk_sb, wv_sb, wo_sb = w_sbs

            cos_sb = cpool.tile([128, WIN], BF16, tag="cos")
            nc.sync.dma_start(cos_sb[:], cosd[:])
            sin_sb = cpool.tile([128, WIN], BF16, tag="sin")
            nc.sync.dma_start(sin_sb[:], sind[:])
            psg_sb = cpool.tile([128, 128], BF16, tag="psg")
            nc.sync.dma_start(psg_sb[:], psgd[:])
            msk_sb = cpool.tile([128, 128], BF16, tag="msk")
            nc.sync.dma_start(msk_sb[:], mskd[:])
            idn_sb = cpool.tile([128, 128], BF16, tag="idn")
            nc.sync.dma_start(idn_sb[:], idnd[:])

            # broadcast APs for the rope tables: free dims (dblock, win, j)
            cos_bc8 = bass.AP(
                tensor=cos_sb.tensor,
                offset=cos_sb.offset,
                ap=[cos_sb.ap[0], [0, 8], [0, MEGA // WIN], [1, WIN]],
            )
            sin_bc1 = bass.AP(
                tensor=sin_sb.tensor,
                offset=sin_sb.offset,
                ap=[sin_sb.ap[0], [0, MEGA // WIN], [1, WIN]],
            )
            msk_bc = bass.AP(
                tensor=msk_sb.tensor,
                offset=msk_sb.offset,
                ap=[msk_sb.ap[0], [0, 4], [1, 128]],
            )

            def megatile(it):
                t0 = it * MEGA

                x_mt = xpool.tile([128, 8, MEGA], BF16, tag="x")
                nc.sync.dma_start(x_mt[:], xt[it])

                # --- Q,K transposed projection + rope ---
                qr_ts = []
                for w_sb, nm in ((wq_sb, "q"), (wk_sb, "k")):
                    qc_t = qkc.tile([128, 8, MEGA], BF16, tag=f"{nm}c")
                    for db in range(8):
                        ps = pp.tile([128, MEGA], F32, tag="pp")
                        for c in range(8):
                            nc.tensor.matmul(
                                ps[:],
                                w_sb[:, c, 128 * db : 128 * db + 128],
                                x_mt[:, c, :],
                                start=(c == 0),
                                stop=(c == 7),
                            )
                        nc.scalar.copy(qc_t[:, db, :], ps[:])
                    qs_t = qks.tile([128, 8, MEGA], BF16, tag=f"{nm}s")
                    for db in range(8):
                        rps = mix.tile([128, MEGA], F32, tag="mix")
                        nc.tensor.matmul(
                            rps[:], psg_sb[:], qc_t[:, db, :],
                            start=True, stop=True,
                        )
                        nc.vector.tensor_mul(qs_t[:, db, :], rps[:], sin_bc1)
                    qr_t = qkr.tile([128, 8, MEGA], BF16, tag=f"{nm}r")
                    nc.vector.tensor_mul(qr_t[:], qc_t[:], cos_bc8)
                    nc.vector.tensor_add(qr_t[:], qr_t[:], qs_t[:])
                    qr_ts.append(qr_t)
                qr_t, kr_t = qr_ts

                if phases < 2:
                    o_t = opool.tile([128, 4, DIM], BF16, tag="o")
                    nc.gpsimd.memset(o_t[:], 0.0)
                    nc.sync.dma_start(out[:, 4 * it : 4 * it + 4, :], o_t[:])
                    return
                # --- V natural projection with ones column ---
                vo_ts = []
                for tb in range(4):
                    vo_t = vpool.tile([128, NHEADS, HDIM + 1], BF16, tag="vo")
                    for hf in range(2):
                        ps = pp.tile([128, MEGA], F32, tag="pp")
                        for c in range(8):
                            nc.tensor.matmul(
                                ps[:],
                                x_mt[:, c, 128 * tb : 128 * tb + 128],
                                wv_sb[:, c, 512 * hf : 512 * hf + 512],
                                start=(c == 0),
                                stop=(c == 7),
                            )
                        nc.scalar.copy(
                            vo_t[:, 8 * hf : 8 * hf + 8, 0:HDIM],
                            ps[:].rearrange("p (h d) -> p h d", h=8),
                        )
                    nc.gpsimd.memset(vo_t[:, :, HDIM : HDIM + 1], 1.0)
                    vo_ts.append(vo_t)

                if phases < 3:
                    o_t = opool.tile([128, 4, DIM], BF16, tag="o")
                    nc.gpsimd.memset(o_t[:], 0.0)
                    nc.sync.dma_start(out[:, 4 * it : 4 * it + 4, :], o_t[:])
                    return
                # --- scores + softmax numerator per head ---
                et_ts = []
                for h in range(NHEADS):
                    po = 64 * (h % 2)
                    ch = h // 2
                    sps = sp.tile([128, 512], F32, tag="sp")
                    for b in range(4):
                        nc.tensor.matmul(
                            sps[:, 128 * b : 128 * b + 128],
                            kr_t[po : po + 64, ch, 128 * b : 128 * b + 128],
                            qr_t[po : po + 64, ch, 128 * b : 128 * b + 128],
                            start=True,
                            stop=True,
                        )
                    et_t = epool.tile([128, 512], BF16, tag="et")
                    nc.scalar.activation(et_t[:], sps[:], AF.Exp, scale=0.125)
                    nc.gpsimd.tensor_mul(et_t[:], et_t[:], msk_bc)
                    et_ts.append(et_t)

                if phases < 4:
                    o_t = opool.tile([128, 4, DIM], BF16, tag="o")
                    nc.gpsimd.memset(o_t[:], 0.0)
                    nc.sync.dma_start(out[:, 4 * it : 4 * it + 4, :], o_t[:])
                    return
                # --- PV + normalize -> attn natural bf16 ---
                attn_ts = []
                for b in range(4):
                    attn_t = apool.tile([128, DIM], BF16, tag="attn")
                    for g in range(4):
                        pa_t = pa.tile([128, 4 * (HDIM + 1)], F32, tag="pa")
                        for h4 in range(4):
                            h = 4 * g + h4
                            nc.tensor.matmul(
                                pa_t[:, 65 * h4 : 65 * h4 + 65],
                                et_ts[h][:, 128 * b : 128 * b + 128],
                                vo_ts[b][:, h, :],
                                start=True,
                                stop=True,
                            )
                        ziv = zpool.tile([128, 4], F32, tag="zi")
                        zsrc = bass.AP(
                            tensor=pa_t.tensor,
                            offset=pa_t.offset + HDIM,
                            ap=[pa_t.ap[0], [HDIM + 1, 4]],
                        )
                        nc.vector.reciprocal(ziv[:], zsrc)
                        srca = bass.AP(
                            tensor=pa_t.tensor,
                            offset=pa_t.offset,
                            ap=[pa_t.ap[0], [HDIM + 1, 4], [1, HDIM]],
                        )
                        zbc = bass.AP(
                            tensor=ziv.tensor,
                            offset=ziv.offset,
                            ap=[ziv.ap[0], [1, 4], [0, HDIM]],
                        )
                        nc.vector.tensor_mul(
                            attn_t[:, 256 * g : 256 * g + 256].rearrange(
                                "p (h d) -> p h d", h=4
                            ),
                            srca,
                            zbc,
                        )
                    attn_ts.append(attn_t)

                if phases < 5:
                    o_t = opool.tile([128, 4, DIM], BF16, tag="o")
                    nc.gpsimd.memset(o_t[:], 0.0)
                    nc.sync.dma_start(out[:, 4 * it : 4 * it + 4, :], o_t[:])
                    return
                # --- attn transpose + WO -> out ---
                o_t = opool.tile([128, 4, DIM], BF16, tag="o")
                for b in range(4):
                    at_t = tpool.tile([128, 8, 128], BF16, tag="at")
                    for qd in range(2):
                        tp = mix.tile([128, 512], BF16, tag="mix")
                        for c4 in range(4):
                            c = 4 * qd + c4
                            nc.tensor.transpose(
                                tp[:, 128 * c4 : 128 * c4 + 128],
                                attn_ts[b][:, 128 * c : 128 * c + 128],
                                idn_sb[:],
                            )
                        nc.scalar.copy(
                            at_t[:, 4 * qd : 4 * qd + 4, :],
                            tp[:].rearrange("p (c t) -> p c t", c=4),
                        )
                    for hf in range(2):
                        ps = pp.tile([128, MEGA], F32, tag="pp")
                        for c in range(8):
                            nc.tensor.matmul(
                                ps[:],
                                at_t[:, c, :],
                                wo_sb[:, c, 512 * hf : 512 * hf + 512],
                                start=(c == 0),
                                stop=(c == 7),
                            )
                        nc.scalar.copy(o_t[:, b, 512 * hf : 512 * hf + 512], ps[:])
                nc.sync.dma_start(out[:, 4 * it : 4 * it + 4, :], o_t[:])

            if use_loop:
                with tc.For_i(
                    0, repeat, 1,
                    hint_engines=(mybir.EngineType.PE,),
                    staggered_reset=staggered,
                ):
                    for it in range(nmega):
                        megatile(it)
            else:
                for _ in range(repeat):
                    for it in range(nmega):
                        megatile(it)
    return nc


_PROGRAMS = {}


def build_program(tokens=TOK_PER_CORE, repeat=1, loop_trips=0, use_loop=True,
                  phases=9, staggered=False):
    # loop_trips kept for interface compat; repeat IS the hardware trip count
    if loop_trips:
        repeat = loop_trips
    key = (tokens, repeat, use_loop, phases, staggered)
    if key not in _PROGRAMS:
        nc = bacc.Bacc("TRN2")
        _emit(nc, tokens, repeat, use_loop, phases, staggered)
        nc.compile()
        _PROGRAMS[key] = nc
    return _PROGRAMS[key]


def host_tables(rope_freqs):
    freqs = np.asarray(rope_freqs, dtype=np.float32)[:WIN]  # [16, 32]
    p = np.arange(128)
    # transposed-layout rope tables [128 (d%32 pattern), 16 (pos)]
    cosT = np.cos(freqs.T[p % 32])  # [128, 16]
    sinT = np.sin(freqs.T[p % 32])
    # signed rotate-half: rh(q)[d] = -q[d+32] (d%64<32), +q[d-32] (else)
    P = np.zeros((128, 128), dtype=np.float32)
    for blk in range(2):
        o = 64 * blk
        for d in range(32):
            P[o + d, o + d + 32] = -1.0
            P[o + d + 32, o + d] = 1.0
    psgT = np.ascontiguousarray(P.T)
    c = np.arange(128)
    msk = (p[:, None] // WIN == c[None, :] // WIN).astype(np.float32)
    idn = np.eye(128, dtype=np.float32)
    return (
        cosT.astype(BF), sinT.astype(BF), psgT.astype(BF),
        np.ascontiguousarray(msk).astype(BF), idn.astype(BF),
    )


def make_in_maps(x, rope_freqs, wq, wk, wv, wo, tokens=TOK_PER_CORE, ncores=NCORES):
    x = np.asarray(x, dtype=np.float32)
    xf = x.reshape(-1, DIM)
    xT = np.ascontiguousarray(xf.T).astype(BF)  # [DIM, TOK_TOTAL]
    nmega = tokens // MEGA
    wqt = np.ascontiguousarray(np.asarray(wq, dtype=np.float32).T).astype(BF)
    wkt = np.ascontiguousarray(np.asarray(wk, dtype=np.float32).T).astype(BF)
    wvt = np.ascontiguousarray(np.asarray(wv, dtype=np.float32).T).astype(BF)
    wot = np.ascontiguousarray(np.asarray(wo, dtype=np.float32).T).astype(BF)
    cosT, sinT, psgT, msk, idn = host_tables(rope_freqs)
    maps = []
    for c in range(ncores):
        sl = slice(c * tokens, (c + 1) * tokens)
        # [DIM, tokens] -> [mega, partition, kchunk, tok]
        xc = (
            xT[:, sl]
            .reshape(8, 128, nmega, MEGA)
            .transpose(2, 1, 0, 3)
        )
        maps.append(
            {
                "xt": np.ascontiguousarray(xc),
                "wqt": wqt,
                "wkt": wkt,
                "wvt": wvt,
                "wot": wot,
                "cosd": cosT,
                "sind": sinT,
                "psgd": psgT,
                "mskd": msk,
                "idnd": idn,
            }
        )
    return maps


def kernel(x, rope_freqs, wq, wk, wv, wo):
    nc = build_program(TOK_PER_CORE, 1)
    maps = make_in_maps(x, rope_freqs, wq, wk, wv, wo)
    res = run_bass_kernel_spmd(nc, maps, core_ids=list(range(NCORES)))
    # out is [128, tokblock, DIM] partition-major; token = tokblock*128 + p
    outs = [
        np.asarray(res.results[c]["out"]).transpose(1, 0, 2).reshape(-1, DIM)
        for c in range(NCORES)
    ]
    full = np.concatenate(outs, axis=0)  # [TOK_TOTAL, DIM] bf16
    return full.astype(np.float32).reshape(B, S, DIM)


# revision 6
# speedup vs baseline: 390.9222x; 1.0107x over previous
"""Trainium2 Bass kernel for LocalWindowAttention (bf16 megatile version).

Reference computation:
  x [B=2, S=8192, D=1024] -> q,k,v = x @ {wq,wk,wv}.T ; RoPE(q,k) with
  positions arange(16) per 16-token window; non-causal softmax attention
  within each window; out = attn @ wo.T.

Strategy: data-parallel over tokens (8 cores x 2048 tokens; windows never
cross shard boundaries).  Per core, 4 megatiles of 512 tokens, everything
bf16 on the wires with f32 PSUM accumulation:

  Q,K are projected directly in TRANSPOSED layout [d, tok] (weight chunk
  stationary, xT moving, N=512) so no PE transpose is needed before the
  scores matmul.  RoPE in transposed layout: rotate-half is a signed
  128x128 block-permutation matmul (P @ QT), then two broadcast DVE
  multiplies (cosT/sinT tables [128, 16] broadcast along (dblock, window))
  and one add.  V is projected in natural layout [tok, d] with a ones
  column appended so the PV matmul also yields the softmax denominator Z.
  Scores per head: 4 diagonal [128,128] blocks packed into one PSUM bank
  -> one Exp (ACT, scale=1/8) -> one same-window mask multiply (Pool).
  PV per (qt-block, 4-head group) -> reciprocal + broadcast normalize ->
  attn natural bf16 -> PE transpose -> WO matmuls -> out (bf16, host
  upcasts).

  The whole body sits inside a hardware For_i loop; `repeat` changes only
  the trip-count immediate, keeping program size constant so the harness's
  wall(r=R)-wall(r=1) delta measures actual execution, not NEFF size.

PSUM bank budget (8 banks): pp(proj+WO) bufs=2, sp(scores) bufs=2,
mix(rh+attnT) bufs=2, pa(PV) bufs=2.  Scores banks alternate head parity
with bufs=2 so each bank only ever sees one K-partition range (mixing
0/64 offsets within a bank crashes the device).
"""

import os
import sys

import numpy as np
import ml_dtypes

for _p in ("/root/.axon_site/_ro/trn_rl_repo", "/opt/trn_rl_repo"):
    if os.path.isdir(_p) and _p not in sys.path:
        sys.path.append(_p)

import concourse.bass as bass
import concourse.tile as tile
from concourse import bacc, mybir
from concourse.bass_utils import run_bass_kernel_spmd

F32 = mybir.dt.float32
BF16 = mybir.dt.bfloat16
AF = mybir.ActivationFunctionType
BF = ml_dtypes.bfloat16

DIM = 1024
NHEADS = 16
HDIM = 64
WIN = 16
B, S = 2, 8192
NCORES = 8
TOK_TOTAL = B * S
TOK_PER_CORE = TOK_TOTAL // NCORES
MEGA = 512  # tokens per megatile


def _emit(nc, tokens, repeat=1, use_loop=True, phases=9, staggered=True):
    nmega = tokens // MEGA

    # x pre-tiled on host: [mega, partition, kchunk, tok] so each partition's
    # per-megatile slice is one contiguous 8KB DMA descriptor
    xt = nc.dram_tensor(
        "xt", [tokens // MEGA, 128, 8, MEGA], BF16, kind="ExternalInput"
    )
    wqt = nc.dram_tensor("wqt", [DIM, DIM], BF16, kind="ExternalInput")
    wkt = nc.dram_tensor("wkt", [DIM, DIM], BF16, kind="ExternalInput")
    wvt = nc.dram_tensor("wvt", [DIM, DIM], BF16, kind="ExternalInput")
    wot = nc.dram_tensor("wot", [DIM, DIM], BF16, kind="ExternalInput")
    cosd = nc.dram_tensor("cosd", [128, WIN], BF16, kind="ExternalInput")
    sind = nc.dram_tensor("sind", [128, WIN], BF16, kind="ExternalInput")
    psgd = nc.dram_tensor("psgd", [128, 128], BF16, kind="ExternalInput")
    mskd = nc.dram_tensor("mskd", [128, 128], BF16, kind="ExternalInput")
    idnd = nc.dram_tensor("idnd", [128, 128], BF16, kind="ExternalInput")
    # out partition-major: [partition, tokblock, d]; host reassembles token
    # order (token = tokblock*128 + partition)
    out = nc.dram_tensor(
        "out", [128, tokens // 128, DIM], BF16, kind="ExternalOutput"
    )
    wrs = [w.rearrange("(c p) o -> p c o", p=128) for w in (wqt, wkt, wvt, wot)]

    with tile.TileContext(nc) as tc:
        with (
            tc.tile_pool(name="wpool", bufs=1) as wpool,
            tc.tile_pool(name="cpool", bufs=1) as cpool,
            tc.tile_pool(name="xpool", bufs=2) as xpool,
            tc.tile_pool(name="qkc", bufs=1) as qkc,
            tc.tile_pool(name="qks", bufs=1) as qks,
            tc.tile_pool(name="qkr", bufs=1) as qkr,
            tc.tile_pool(name="vpool", bufs=4) as vpool,
            tc.tile_pool(name="epool", bufs=16) as epool,
            tc.tile_pool(name="zpool", bufs=4) as zpool,
            tc.tile_pool(name="apool", bufs=4) as apool,
            tc.tile_pool(name="tpool", bufs=2) as tpool,
            tc.tile_pool(name="opool", bufs=2) as opool,
            tc.tile_pool(name="pp", bufs=2, space="PSUM") as pp,
            tc.tile_pool(name="sp", bufs=2, space="PSUM") as sp,
            tc.tile_pool(name="mix", bufs=2, space="PSUM") as mix,
            tc.tile_pool(name="pa", bufs=2, space="PSUM") as pa,
        ):
            w_sbs = []
            for i, wr in enumerate(wrs):
                w_sb = wpool.tile([128, 8, DIM], BF16, tag=f"w{i}")
                nc.sync.dma_start(w_sb[:], wr[:])
                w_sbs.append(w_sb)
            wq_sb, wk_sb, wv_sb, wo_sb = w_sbs

            cos_sb = cpool.tile([128, WIN], BF16, tag="cos")
            nc.sync.dma_start(cos_sb[:], cosd[:])
            sin_sb = cpool.tile([128, WIN], BF16, tag="sin")
            nc.sync.dma_start(sin_sb[:], sind[:])
            psg_sb = cpool.tile([128, 128], BF16, tag="psg")
            nc.sync.dma_start(psg_sb[:], psgd[:])
            msk_sb = cpool.tile([128, 128], BF16, tag="msk")
            nc.sync.dma_start(msk_sb[:], mskd[:])
            idn_sb = cpool.tile([128, 128], BF16, tag="idn")
            nc.sync.dma_start(idn_sb[:], idnd[:])

            # broadcast APs for the rope tables: free dims (dblock, win, j)
            cos_bc8 = bass.AP(
                tensor=cos_sb.tensor,
                offset=cos_sb.offset,
                ap=[cos_sb.ap[0], [0, 8], [0, MEGA // WIN], [1, WIN]],
            )
            sin_bc1 = bass.AP(
                tensor=sin_sb.tensor,
                offset=sin_sb.offset,
                ap=[sin_sb.ap[0], [0, MEGA // WIN], [1, WIN]],
            )
            msk_bc = bass.AP(
                tensor=msk_sb.tensor,
                offset=msk_sb.offset,
                ap=[msk_sb.ap[0], [0, 4], [1, 128]],
            )

            def megatile(it):
                t0 = it * MEGA

                x_mt = xpool.tile([128, 8, MEGA], BF16, tag="x")
                nc.sync.dma_start(x_mt[:], xt[it])

                # --- Q,K transposed projection + rope ---
                qr_ts = []
                for w_sb, nm in ((wq_sb, "q"), (wk_sb, "k")):
                    qc_t = qkc.tile([128, 8, MEGA], BF16, tag=f"{nm}c")
                    for db in range(8):
                        ps = pp.tile([128, MEGA], F32, tag="pp")
                        for c in range(8):
                            nc.tensor.matmul(
                                ps[:],
                                w_sb[:, c, 128 * db : 128 * db + 128],
                                x_mt[:, c, :],
                                start=(c == 0),
                                stop=(c == 7),
                            )
                        nc.scalar.copy(qc_t[:, db, :], ps[:])
                    qs_t = qks.tile([128, 8, MEGA], BF16, tag=f"{nm}s")
                    for db in range(8):
                        rps = mix.tile([128, MEGA], F32, tag="mix")
                        nc.tensor.matmul(
                            rps[:], psg_sb[:], qc_t[:, db, :],
                            start=True, stop=True,
                        )
                        nc.vector.tensor_mul(qs_t[:, db, :], rps[:], sin_bc1)
                    qr_t = qkr.tile([128, 8, MEGA], BF16, tag=f"{nm}r")
                    nc.vector.tensor_mul(qr_t[:], qc_t[:], cos_bc8)
                    nc.vector.tensor_add(qr_t[:], qr_t[:], qs_t[:])
                    qr_ts.append(qr_t)
                qr_t, kr_t = qr_ts

                if phases < 2:
                    o_t = opool.tile([128, 4, DIM], BF16, tag="o")
                    nc.gpsimd.memset(o_t[:], 0.0)
                    nc.sync.dma_start(out[:, 4 * it : 4 * it + 4, :], o_t[:])
                    return
                # --- V natural projection with ones column ---
                vo_ts = []
                for tb in range(4):
                    vo_t = vpool.tile([128, NHEADS, HDIM + 1], BF16, tag="vo")
                    for hf in range(2):
                        ps = pp.tile([128, MEGA], F32, tag="pp")
                        for c in range(8):
                            nc.tensor.matmul(
                                ps[:],
                                x_mt[:, c, 128 * tb : 128 * tb + 128],
                                wv_sb[:, c, 512 * hf : 512 * hf + 512],
                                start=(c == 0),
                                stop=(c == 7),
                            )
                        nc.scalar.copy(
                            vo_t[:, 8 * hf : 8 * hf + 8, 0:HDIM],
                            ps[:].rearrange("p (h d) -> p h d", h=8),
                        )
                    nc.gpsimd.memset(vo_t[:, :, HDIM : HDIM + 1], 1.0)
                    vo_ts.append(vo_t)

                if phases < 3:
                    o_t = opool.tile([128, 4, DIM], BF16, tag="o")
                    nc.gpsimd.memset(o_t[:], 0.0)
                    nc.sync.dma_start(out[:, 4 * it : 4 * it + 4, :], o_t[:])
                    return
                # --- scores + softmax numerator per head ---
                et_ts = []
                for h in range(NHEADS):
                    po = 64 * (h % 2)
                    ch = h // 2
                    sps = sp.tile([128, 512], F32, tag="sp")
                    for b in range(4):
                        nc.tensor.matmul(
                            sps[:, 128 * b : 128 * b + 128],
                            kr_t[po : po + 64, ch, 128 * b : 128 * b + 128],
                            qr_t[po : po + 64, ch, 128 * b : 128 * b + 128],
                            start=True,
                            stop=True,
                        )
                    et_t = epool.tile([128, 512], BF16, tag="et")
                    nc.scalar.activation(et_t[:], sps[:], AF.Exp, scale=0.125)
                    nc.gpsimd.tensor_mul(et_t[:], et_t[:], msk_bc)
                    et_ts.append(et_t)

                if phases < 4:
                    o_t = opool.tile([128, 4, DIM], BF16, tag="o")
                    nc.gpsimd.memset(o_t[:], 0.0)
                    nc.sync.dma_start(out[:, 4 * it : 4 * it + 4, :], o_t[:])
                    return
                # --- PV + normalize -> attn natural bf16 ---
                attn_ts = []
                for b in range(4):
                    attn_t = apool.tile([128, DIM], BF16, tag="attn")
                    for g in range(4):
                        pa_t = pa.tile([128, 4 * (HDIM + 1)], F32, tag="pa")
                        for h4 in range(4):
                            h = 4 * g + h4
                            nc.tensor.matmul(
                                pa_t[:, 65 * h4 : 65 * h4 + 65],
                                et_ts[h][:, 128 * b : 128 * b + 128],
                                vo_ts[b][:, h, :],
                                start=True,
                                stop=True,
                            )
                        ziv = zpool.tile([128, 4], F32, tag="zi")
                        zsrc = bass.AP(
                            tensor=pa_t.tensor,
                            offset=pa_t.offset + HDIM,
                            ap=[pa_t.ap[0], [HDIM + 1, 4]],
                        )
                        nc.vector.reciprocal(ziv[:], zsrc)
                        srca = bass.AP(
                            tensor=pa_t.tensor,
                            offset=pa_t.offset,
                            ap=[pa_t.ap[0], [HDIM + 1, 4], [1, HDIM]],
                        )
                        zbc = bass.AP(
                            tensor=ziv.tensor,
                            offset=ziv.offset,
                            ap=[ziv.ap[0], [1, 4], [0, HDIM]],
                        )
                        nc.vector.tensor_mul(
                            attn_t[:, 256 * g : 256 * g + 256].rearrange(
                                "p (h d) -> p h d", h=4
                            ),
                            srca,
                            zbc,
                        )
                    attn_ts.append(attn_t)

                if phases < 5:
                    o_t = opool.tile([128, 4, DIM], BF16, tag="o")
                    nc.gpsimd.memset(o_t[:], 0.0)
                    nc.sync.dma_start(out[:, 4 * it : 4 * it + 4, :], o_t[:])
                    return
                # --- attn transpose + WO -> out ---
                o_t = opool.tile([128, 4, DIM], BF16, tag="o")
                for b in range(4):
                    at_t = tpool.tile([128, 8, 128], BF16, tag="at")
                    for qd in range(2):
                        tp = mix.tile([128, 512], BF16, tag="mix")
                        for c4 in range(4):
                            c = 4 * qd + c4
                            nc.tensor.transpose(
                                tp[:, 128 * c4 : 128 * c4 + 128],
                                attn_ts[b][:, 128 * c : 128 * c + 128],
                                idn_sb[:],
                            )
                        nc.scalar.copy(
                            at_t[:, 4 * qd : 4 * qd + 4, :],
                            tp[:].rearrange("p (c t) -> p c t", c=4),
                        )
                    for hf in range(2):
                        ps = pp.tile([128, MEGA], F32, tag="pp")
                        for c in range(8):
                            nc.tensor.matmul(
                                ps[:],
                                at_t[:, c, :],
                                wo_sb[:, c, 512 * hf : 512 * hf + 512],
                                start=(c == 0),
                                stop=(c == 7),
                            )
                        nc.scalar.copy(o_t[:, b, 512 * hf : 512 * hf + 512], ps[:])
                nc.sync.dma_start(out[:, 4 * it : 4 * it + 4, :], o_t[:])

            if use_loop:
                with tc.For_i(
                    0, repeat, 1,
                    hint_engines=(mybir.EngineType.PE,),
                    staggered_reset=staggered,
                ):
                    for it in range(nmega):
                        megatile(it)
            else:
                for _ in range(repeat):
                    for it in range(nmega):
                        megatile(it)
    return nc


_PROGRAMS = {}


def build_program(tokens=TOK_PER_CORE, repeat=1, loop_trips=0, use_loop=True,
                  phases=9, staggered=True):
    # loop_trips kept for interface compat; repeat IS the hardware trip count
    if loop_trips:
        repeat = loop_trips
    key = (tokens, repeat, use_loop, phases, staggered)
    if key not in _PROGRAMS:
        nc = bacc.Bacc("TRN2")
        _emit(nc, tokens, repeat, use_loop, phases, staggered)
        nc.compile()
        _PROGRAMS[key] = nc
    return _PROGRAMS[key]


def host_tables(rope_freqs):
    freqs = np.asarray(rope_freqs, dtype=np.float32)[:WIN]  # [16, 32]
    p = np.arange(128)
    # transposed-layout rope tables [128 (d%32 pattern), 16 (pos)]
    cosT = np.cos(freqs.T[p % 32])  # [128, 16]
    sinT = np.sin(freqs.T[p % 32])
    # signed rotate-half: rh(q)[d] = -q[d+32] (d%64<32), +q[d-32] (else)
    P = np.zeros((128, 128), dtype=np.float32)
    for blk in range(2):
        o = 64 * blk
        for d in range(32):
            P[o + d, o + d + 32] = -1.0
            P[o + d + 32, o + d] = 1.0
    psgT = np.ascontiguousarray(P.T)
    c = np.arange(128)
    msk = (p[:, None] // WIN == c[None, :] // WIN).astype(np.float32)
    idn = np.eye(128, dtype=np.float32)
    return (
        cosT.astype(BF), sinT.astype(BF), psgT.astype(BF),
        np.ascontiguousarray(msk).astype(BF), idn.astype(BF),
    )


def make_in_maps(x, rope_freqs, wq, wk, wv, wo, tokens=TOK_PER_CORE, ncores=NCORES):
    x = np.asarray(x, dtype=np.float32)
    xf = x.reshape(-1, DIM)
    xT = np.ascontiguousarray(xf.T).astype(BF)  # [DIM, TOK_TOTAL]
    nmega = tokens // MEGA
    wqt = np.ascontiguousarray(np.asarray(wq, dtype=np.float32).T).astype(BF)
    wkt = np.ascontiguousarray(np.asarray(wk, dtype=np.float32).T).astype(BF)
    wvt = np.ascontiguousarray(np.asarray(wv, dtype=np.float32).T).astype(BF)
    wot = np.ascontiguousarray(np.asarray(wo, dtype=np.float32).T).astype(BF)
    cosT, sinT, psgT, msk, idn = host_tables(rope_freqs)
    maps = []
    for c in range(ncores):
        sl = slice(c * tokens, (c + 1) * tokens)
        # [DIM, tokens] -> [mega, partition, kchunk, tok]
        xc = (
            xT[:, sl]
            .reshape(8, 128, nmega, MEGA)
            .transpose(2, 1, 0, 3)
        )
        maps.append(
            {
                "xt": np.ascontiguousarray(xc),
                "wqt": wqt,
                "wkt": wkt,
                "wvt": wvt,
                "wot": wot,
                "cosd": cosT,
                "sind": sinT,
                "psgd": psgT,
                "mskd": msk,
                "idnd": idn,
            }
        )
    return maps


def kernel(x, rope_freqs, wq, wk, wv, wo):
    nc = build_program(TOK_PER_CORE, 1)
    maps = make_in_maps(x, rope_freqs, wq, wk, wv, wo)
    res = run_bass_kernel_spmd(nc, maps, core_ids=list(range(NCORES)))
    # out is [128, tokblock, DIM] partition-major; token = tokblock*128 + p
    outs = [
        np.asarray(res.results[c]["out"]).transpose(1, 0, 2).reshape(-1, DIM)
        for c in range(NCORES)
    ]
    full = np.concatenate(outs, axis=0)  # [TOK_TOTAL, DIM] bf16
    return full.astype(np.float32).reshape(B, S, DIM)
